# revision 45
# baseline (speedup 1.0000x reference)
"""DCRNN (2-layer encoder/decoder DCGRU, N=512 nodes, B=32, U=64, K=2, 2 supports)
Trainium2 Bass/Tile kernel, data-parallel over batch across 8 NeuronCores.

Key reformulation: the Chebyshev graph conv
    gconv(X) = sum_m T_m(S) @ X @ W_m,  T in {I, S_A, 2S_A^2-I, S_B, 2S_B^2-I}
with the T_m precomputed on host. Per gconv:
  stage 1 (dense):     A_m = X @ W_m      -- activations kept channel-on-partition
                       (X^T used as matmul lhsT, W as rhs -> A node-on-partition)
  stage 2 (diffusion): out^T = X @ W_0 + sum_{m>0} (T_m A_m)^T -- A_m as lhsT,
                       T_m^T as rhs, accumulated over m in PSUM; bias+sigmoid/
                       tanh fused into the PSUM->SBUF move.
This needs no tensor transposes at runtime. All weights/supports/state resident
in SBUF for the whole 24-step recurrence; only encoder inputs in / decoder
outputs out touch HBM.

All matmul operands are bf16 (PSUM accumulation stays fp32): fp32r stationary
operands disable the PE's fast-weight-load path, serializing an LDWEIGHTS
(~220ns) with every MATMUL; bf16 streams at the same 1 cycle/row but lets the
weight load overlap. bf16 state also doubles DVE elementwise throughput.

Scheduling notes (from HW traces): engine queues are strictly in-order, so
the program order must avoid head-of-line blocking -- stage-1 for all four
local batches runs before any stage-2 so the PSUM->SBUF drains have matmul
cover; the decoder projection runs after BOTH layer-1 candidate phases so its
wait on the GRU tail doesn't block ready matmuls; layer-0's new hidden state
is written directly into the layer-1 input tile (its immediate consumer) with
the other copies pushed off the critical path onto gpsimd.
"""

import sys

sys.path.insert(0, "/opt/trn_rl_repo")

import numpy as np

import concourse.bass as bass
import concourse.mybir as mybir
import concourse.tile as tile
from concourse import bacc, bass_utils

# Model dims (fixed by the problem)
N = 512
T_ENC = 12
HOR = 12
U = 64
NM = 5  # num diffusion matrices (I + 2 per support * 2 supports)
B = 32
NCORES = 8
BL = B // NCORES  # local batch = 4
BI = BL * N  # 2048: the (b, node) free dim
C0 = 1 + U  # 65 input channels, layer 0
C1 = U + U  # 128 input channels, layer 1
KCH = N // 128  # 4 node chunks

F32 = mybir.dt.float32
BF16 = mybir.dt.bfloat16
AF = mybir.ActivationFunctionType


def _build_program(n_enc=T_ENC, n_dec=HOR):
    nc = bacc.Bacc("TRN2", target_bir_lowering=False, debug=False)

    # ---- DRAM I/O ----
    d_xenc = nc.dram_tensor("xenc", [n_enc, BI], BF16, kind="ExternalInput")
    # diffusion matrices m=1..4 (identity term is folded into stage 2)
    d_tm = nc.dram_tensor("tmats", [(NM - 1) * KCH * 128, 512], BF16,
                          kind="ExternalInput")
    d_w = {}
    for pfx in ("e", "d"):
        for lyr, c_in in ((0, C0), (1, C1)):
            d_w[f"{pfx}wg{lyr}"] = nc.dram_tensor(
                f"{pfx}wg{lyr}", [c_in, NM * 2 * U], BF16, kind="ExternalInput"
            )
            d_w[f"{pfx}wc{lyr}"] = nc.dram_tensor(
                f"{pfx}wc{lyr}", [c_in, NM * U], BF16, kind="ExternalInput"
            )
            d_w[f"{pfx}bgru{lyr}"] = nc.dram_tensor(
                f"{pfx}bgru{lyr}", [2 * U, 1], F32, kind="ExternalInput"
            )
            d_w[f"{pfx}bc{lyr}"] = nc.dram_tensor(
                f"{pfx}bc{lyr}", [2 * U, 1], F32, kind="ExternalInput"
            )
    d_pw = nc.dram_tensor("pw", [U, 2], BF16, kind="ExternalInput")
    d_pb = nc.dram_tensor("pb", [1, 1], F32, kind="ExternalInput")
    d_out = nc.dram_tensor("outs", [n_dec, BI], BF16, kind="ExternalOutput")

    with tile.TileContext(nc) as tc:
        _body(tc, n_enc, n_dec, d_xenc, d_tm, d_w, d_pw, d_pb, d_out)
    nc.compile()
    return nc


def _body(tc, n_enc, n_dec, d_xenc, d_tm, d_w, d_pw, d_pb, d_out):
    nc = tc.nc
    consts = tc.alloc_tile_pool(name="consts", bufs=1)
    work = tc.alloc_tile_pool(name="work", bufs=1)
    gpool = tc.alloc_tile_pool(name="gpool", bufs=2)
    # >=16 ag / >=8 ac bufs so a full cell's stage-1 tiles are live at once
    # (fewer forces stage-1 drains to WAR-wait on stage-2's reads)
    ag_pool = tc.alloc_tile_pool(name="agp", bufs=18)
    ac_pool = tc.alloc_tile_pool(name="acp", bufs=10)
    # 6 stage-1 PSUM banks let the 16-matmul stage-1 burst run ~6 deep ahead
    # of the scalar/vector drains (687/551ns each vs 216ns matmul issue); the
    # drains catch up during the drain-free stage-2 window. stage-2 accs
    # only need 2 in flight (acts on acc N overlap matmuls of acc N+1).
    ps1 = tc.alloc_tile_pool(name="ps1", bufs=6, space="PSUM")
    ps2 = tc.alloc_tile_pool(name="ps2", bufs=2, space="PSUM")

    # ---- resident constants ----
    # one tile per (m, k) chunk so early stage-2 consumers wait only on their
    # own chunk's DMA, not the whole 2.1 MB load (DMAs issued after the
    # barrier below so the first gate matmuls don't wait for them)
    tm_sb = {}
    for m in range(1, NM):
        for k in range(KCH):
            tm_sb[(m, k)] = consts.tile([128, 512], BF16, name=f"tm_{m}_{k}")

    w_sb = {}
    for key, dt in d_w.items():
        w_sb[key] = consts.tile(list(dt.shape), dt.dtype, name=f"sb_{key}")
    pw_sb = consts.tile([U, 2], BF16, name="pw_sb")
    pb_sb = consts.tile([1, 1], F32, name="pb_sb")

    def load_weights(pfx):
        for key, dt in d_w.items():
            if key[0] == pfx:
                nc.sync.dma_start(out=w_sb[key][:, :], in_=dt[:, :])

    # ---- persistent state (channel-on-partition, free dim = (b, node)) ----
    X0 = work.tile([C0, BI], BF16, name="X0")  # [h0 ; x]
    X0c = work.tile([C0, BI], BF16, name="X0c")  # [r*h0 ; x]
    X1 = work.tile([C1, BI], BF16, name="X1")  # [h0 ; h1]
    X1c = work.tile([C1, BI], BF16, name="X1c")  # [r*h1 ; h0]
    h1t = work.tile([U, BI], BF16, name="h1t")  # h1 at partition base 0

    nc.vector.memset(X0[0:U, :], 0.0)
    nc.vector.memset(X1c[U:C1, :], 0.0)
    nc.gpsimd.memset(X1[U:C1, :], 0.0)
    nc.vector.memset(h1t[:, :], 0.0)

    # collapse the enc-weight/init dependencies into one semaphore so the
    # first consumers don't exceed per-instruction sync-wait slots; decoder
    # weights and the tm chunks ride their own DMA deps
    tc.strict_bb_all_engine_barrier()

    load_weights("e")
    for m in range(1, NM):
        for k in range(KCH):
            row = ((m - 1) * KCH + k) * 128
            # ride the idle gpsimd queue: its DMA completion counter is
            # separate from the sync queue's, so the first gate matmuls
            # (waiting on enc weights via sync) don't also wait these 2.1 MB
            nc.gpsimd.dma_start(out=tm_sb[(m, k)][:, :], in_=d_tm[row : row + 128, :])

    def cell_phases(X, Xc, c_in, rh_lo, h_src, h64_src, wg, bgru, wc, bc,
                    h_writer, post):
        """One DCGRU cell split into stage-1/stage-2 phases, each over
        batch-pair p in {0, 1}."""
        # gate outputs in one [u ; r] tile (weight columns permuted on host):
        # u at base 0 pairs with h_src (base 0); r at base 64 pairs with the
        # resident base-64 h copy (TensorTensor needs equal input bases)
        RU = gpool.tile([2 * U, BI], BF16, tag="RU", name="RU", bufs=1)
        # (u-1) duplicated at base 0 and base 64 so the per-half wc muls
        # against the fused-tanh output have base-aligned inputs
        Um1b = gpool.tile([2 * U, BI], BF16, tag="Um1b", name="Um1b", bufs=1)
        uh = gpool.tile([U, BI], BF16, tag="uh", name="uh", bufs=1)
        Ct2 = gpool.tile([2 * U, BI // 2], BF16, tag="Ct2", name="Ct2", bufs=1)
        wc_t = gpool.tile([U, BI], BF16, tag="wct", name="wc_t", bufs=1)
        ag = {}
        ac = {}

        def gate_s1(p):
            # stage 1: A_m = X @ Wg_m for m=1..4 (m=0 folded into stage 2)
            for b in (2 * p, 2 * p + 1):
                for k in range(KCH):
                    pg = ps1.tile([128, 512], F32, tag="s1", name="pg")
                    lhsT = X[0:c_in, b * N + k * 128 : b * N + (k + 1) * 128]
                    nc.tensor.matmul(
                        pg, lhsT, wg[:, 128:640], start=True, stop=True
                    )
                    a = ag_pool.tile([128, 4 * 128], BF16, tag="ag", name="ag")
                    ag[(b, k)] = a
                    # split PSUM->SBUF drains between scalar and vector so
                    # neither engine serializes behind the whole batch
                    if k % 2 == 0:
                        nc.scalar.copy(out=a[:, :], in_=pg)
                    else:
                        nc.vector.tensor_copy(out=a[:, :], in_=pg)

        def gate_s2(p):
            # stage 2: acc = X @ Wg_0 + sum_{m>0} (T_m A_m)^T, fused sigmoid
            for b in (2 * p, 2 * p + 1):
                acc = ps2.tile([128, 512], F32, tag="s2", name="accg")
                nc.tensor.matmul(
                    acc,
                    wg[:, 0:128],
                    X[0:c_in, b * N : (b + 1) * N],
                    start=True,
                    stop=False,
                )
                for m in range(1, NM):
                    for k in range(KCH):
                        nc.tensor.matmul(
                            acc,
                            ag[(b, k)][:, (m - 1) * 128 : m * 128],
                            tm_sb[(m, k)],
                            start=False,
                            stop=(m == NM - 1 and k == KCH - 1),
                        )
                bcols = slice(b * N, (b + 1) * N)
                nc.scalar.activation(
                    out=RU[:, bcols], in_=acc[0 : 2 * U, :], func=AF.Sigmoid,
                    bias=bgru[:, 0:1], scale=1.0,
                )
            pcols = slice(2 * p * N, 2 * (p + 1) * N)
            # r*h -> candidate input rows; u*h for the GRU blend; (u-1) for
            # the blend's candidate half, computed off the critical tail
            nc.vector.tensor_mul(
                out=Xc[rh_lo : rh_lo + U, pcols],
                in0=RU[U : 2 * U, pcols],
                in1=h64_src[:, pcols],
            )
            nc.vector.tensor_mul(
                out=uh[:, pcols],
                in0=RU[0:U, pcols],
                in1=h_src[:, pcols],
            )
            nc.vector.tensor_scalar_sub(Um1b[0:U, pcols], RU[0:U, pcols], 1.0)
            nc.vector.tensor_scalar_sub(Um1b[U : 2 * U, pcols], RU[0:U, pcols], 1.0)

        def cand_s1(p):
            # stage 1: A_m^c = Xc @ Wc_m for m=1..4 (m=0 folded into stage 2)
            for b in (2 * p, 2 * p + 1):
                for k in range(KCH):
                    pc = ps1.tile([128, 512], F32, tag="s1", name="pc")
                    lhsT = Xc[0:c_in, b * N + k * 128 : b * N + (k + 1) * 128]
                    nc.tensor.matmul(
                        pc[:, 0:256], lhsT, wc[:, U:], start=True, stop=True
                    )
                    if b % 2 == 0:
                        a = ac_pool.tile([128, NM - 1, 2, U], BF16, tag="ac", name="ac")
                        ac[(p, k)] = a
                    dst = ac[(p, k)][:, :, b % 2, :]
                    src_v = pc[:, 0:256].rearrange("p (m u) -> p m u", m=NM - 1)
                    if k % 2 == 0:
                        nc.scalar.copy(out=dst, in_=src_v)
                    else:
                        nc.vector.tensor_copy(out=dst, in_=src_v)

        def cand_s2(p):
            acc = ps2.tile([128, 512], F32, tag="s2", name="accc")
            # m=0 (identity diffusion) comes straight from Xc @ Wc_0, one
            # 64-partition half of the accumulator per batch
            for half in range(2):
                b = 2 * p + half
                nc.tensor.matmul(
                    acc[half * U : (half + 1) * U, :],
                    wc[:, 0:U],
                    Xc[0:c_in, b * N : (b + 1) * N],
                    start=True,
                    stop=False,
                    skip_group_check=True,
                )
            for m in range(1, NM):
                for k in range(KCH):
                    nc.tensor.matmul(
                        acc,
                        ac[(p, k)][:, m - 1, :, :],
                        tm_sb[(m, k)],
                        start=False,
                        stop=(m == NM - 1 and k == KCH - 1),
                        skip_group_check=True,
                    )
            pc512 = slice(p * 512, (p + 1) * 512)
            nc.scalar.activation(
                out=Ct2[:, pc512],
                in_=acc[0 : 2 * U, :],
                func=AF.Tanh,
                bias=bc[:, 0:1],
                scale=1.0,
            )
            pcols = slice(2 * p * N, 2 * (p + 1) * N)
            # h_new = u*h + (1-u)*c == u*h - (u-1)*c
            for half in range(2):
                b = 2 * p + half
                bcols = slice(b * N, (b + 1) * N)
                nc.vector.tensor_mul(
                    out=wc_t[:, bcols],
                    in0=Um1b[half * U : (half + 1) * U, bcols],
                    in1=Ct2[half * U : (half + 1) * U, pc512],
                )
            nc.vector.tensor_sub(
                out=h_writer(p, pcols), in0=uh[:, pcols], in1=wc_t[:, pcols]
            )
            post(p, pcols)

        return gate_s1, gate_s2, cand_s1, cand_s2

    def l0_writer(p, pcols):
        # write h0 straight into its immediate consumer (layer-1 gate lhsT);
        # the copies for next step / candidate input happen off the critical
        # path on gpsimd
        return X1[0:U, pcols]

    def _copy(out, in_):
        # same-dtype COPY (and anything on gpsimd) runs several times slower
        # than the DVE tensor-scalar path; express state copies as +0.0 on
        # vector
        nc.vector.tensor_scalar_add(out, in_, 0.0)

    def l0_post(p, pcols):
        _copy(X1c[U:C1, pcols], X1[0:U, pcols])
        _copy(X0[0:U, pcols], X1[0:U, pcols])

    def l1_writer(p, pcols):
        return h1t[:, pcols]

    def l1_post(p, pcols):
        _copy(X1[U:C1, pcols], h1t[:, pcols])

    def proj_phase(p):
        # projection for pair p: out = h1 . pw + pb -> feeds back as x row.
        # pp tiles come from ps1 (idle during the projection) so both
        # quarters' matmuls can be in flight; the X0-row acts go first since
        # the next step's gate m=0 matmuls consume them before X0c is needed.
        pps = {}
        for q in (2 * p, 2 * p + 1):
            pp = ps1.tile([2, 512], F32, tag="s1", name="pp")
            nc.tensor.matmul(
                pp,
                pw_sb[:, 0:2],
                h1t[:, q * 512 : (q + 1) * 512],
                start=True,
                stop=True,
            )
            pps[q] = pp
        for q in (2 * p, 2 * p + 1):
            nc.scalar.activation(
                out=X0[U:C0, q * 512 : (q + 1) * 512],
                in_=pps[q][0:1, :],
                func=AF.Identity,
                bias=pb_sb[:, 0:1],
                scale=1.0,
            )
        for q in (2 * p, 2 * p + 1):
            nc.scalar.activation(
                out=X0c[U:C0, q * 512 : (q + 1) * 512],
                in_=pps[q][0:1, :],
                func=AF.Identity,
                bias=pb_sb[:, 0:1],
                scale=1.0,
            )

    def run_step(pfx, dec_t=None):
        gs1_0, gs2_0, cs1_0, cs2_0 = cell_phases(
            X0, X0c, C0, 0, X0[0:U, :], X1c[U:C1, :],
            w_sb[f"{pfx}wg0"], w_sb[f"{pfx}bgru0"],
            w_sb[f"{pfx}wc0"], w_sb[f"{pfx}bc0"],
            l0_writer, l0_post,
        )
        gs1_1, gs2_1, cs1_1, cs2_1 = cell_phases(
            X1, X1c, C1, 0, h1t[:, :], X1[U:C1, :],
            w_sb[f"{pfx}wg1"], w_sb[f"{pfx}bgru1"],
            w_sb[f"{pfx}wc1"], w_sb[f"{pfx}bc1"],
            l1_writer, l1_post,
        )
        gs1_0(0); gs1_0(1); gs2_0(0); gs2_0(1)
        cs1_0(0); cs1_0(1); cs2_0(0); cs2_0(1)
        gs1_1(0); gs1_1(1); gs2_1(0); gs2_1(1)
        cs1_1(0); cs1_1(1); cs2_1(0); cs2_1(1)
        if dec_t is not None:
            proj_phase(0)
            proj_phase(1)
            nc.sync.dma_start(out=d_out[dec_t : dec_t + 1, :], in_=X0[U:C0, :])

    # ================= encoder =================
    for t in range(n_enc):
        nc.sync.dma_start(out=X0[U:C0, :], in_=d_xenc[t : t + 1, :])
        nc.sync.dma_start(out=X0c[U:C0, :], in_=d_xenc[t : t + 1, :])
        run_step("e")
        if t == 0:
            # stream decoder weights in during encoder compute; the sync
            # queue is idle from here until the next step's input DMA
            load_weights("d")
            nc.sync.dma_start(out=pw_sb[:, :], in_=d_pw[:, :])
            nc.sync.dma_start(out=pb_sb, in_=d_pb[:, :])

    # ================= decoder =================
    nc.gpsimd.memset(X0[U:C0, :], 0.0)
    nc.gpsimd.memset(X0c[U:C0, :], 0.0)
    for t in range(n_dec):
        run_step("d", dec_t=t)

    for pool in (ps2, ps1, ac_pool, ag_pool, gpool, work, consts):
        pool.release()


# --------------------------------------------------------------------------
# host-side packing
# --------------------------------------------------------------------------
def _prep_shared(inputs):
    import ml_dtypes

    bf16 = ml_dtypes.bfloat16
    sup = np.asarray(inputs["supports"], np.float64)
    eye = np.eye(N, dtype=np.float64)
    tms = [
        sup[0],
        2.0 * (sup[0] @ sup[0]) - eye,
        sup[1],
        2.0 * (sup[1] @ sup[1]) - eye,
    ]
    tmats = np.stack([t.T for t in tms]).astype(np.float32)  # [m-1, j, i]
    tmats = tmats.reshape((NM - 1) * KCH * 128, 512)

    shared = {"tmats": np.ascontiguousarray(tmats.astype(bf16))}
    for pfx, name in (("e", "enc"), ("d", "dec")):
        for lyr, c_in in ((0, C0), (1, C1)):
            wg = np.asarray(inputs[f"{name}{lyr}_Wg"], np.float32)
            wc = np.asarray(inputs[f"{name}{lyr}_Wc"], np.float32)
            wg = wg.reshape(c_in, NM * 2 * U)
            wc = wc.reshape(c_in, NM * U)
            bg = np.asarray(inputs[f"{name}{lyr}_bg"], np.float32)
            bc = np.asarray(inputs[f"{name}{lyr}_bc"], np.float32)
            perm_ur = np.r_[U : 2 * U, 0:U]  # gate out-channels as [u ; r]
            wg_r = wg.reshape(c_in, NM, 2 * U)[:, :, perm_ur].reshape(
                c_in, NM * 2 * U
            )
            wc_r = wc.reshape(c_in, NM, U).reshape(c_in, NM * U)
            if lyr == 0:
                perm = np.r_[1:c_in, 0]  # rows [h..., x]
                wg_r = wg_r[perm]
                wc_r = wc_r[perm]
            else:
                # X1c rows are [r*h1 ; h0]: candidate weight rows follow
                wc_r = wc_r[np.r_[U:c_in, 0:U]]
            shared[f"{pfx}wg{lyr}"] = np.ascontiguousarray(wg_r.astype(bf16))
            shared[f"{pfx}wc{lyr}"] = np.ascontiguousarray(wc_r.astype(bf16))
            shared[f"{pfx}bgru{lyr}"] = np.ascontiguousarray(
                np.concatenate([bg[U:], bg[:U]]).reshape(2 * U, 1)
            )
            shared[f"{pfx}bc{lyr}"] = np.ascontiguousarray(
                np.concatenate([bc, bc]).reshape(2 * U, 1)
            )
    pw = np.asarray(inputs["proj_W"], np.float32).reshape(U, 1)
    shared["pw"] = np.ascontiguousarray(
        np.concatenate([pw, np.zeros((U, 1), np.float32)], axis=1).astype(bf16)
    )
    shared["pb"] = np.asarray(inputs["proj_b"], np.float32).reshape(1, 1)
    return shared


def _make_in_maps(inputs, n_enc=T_ENC):
    import ml_dtypes

    shared = _prep_shared(inputs)
    x = np.asarray(inputs["inputs"], np.float32)  # (T, B, N)
    in_maps = []
    for c in range(NCORES):
        m = dict(shared)
        m["xenc"] = np.ascontiguousarray(
            x[:n_enc, c * BL : (c + 1) * BL, :]
            .reshape(n_enc, BI)
            .astype(ml_dtypes.bfloat16)
        )
        in_maps.append(m)
    return in_maps


_PROG_CACHE = {}


def _get_program(n_enc=T_ENC, n_dec=HOR):
    key = (n_enc, n_dec)
    if key not in _PROG_CACHE:
        _PROG_CACHE[key] = _build_program(n_enc, n_dec)
    return _PROG_CACHE[key]


def _run(inputs, n_enc=T_ENC, n_dec=HOR, **kw):
    nc = _get_program(n_enc, n_dec)
    in_maps = _make_in_maps(inputs, n_enc)
    res = bass_utils.run_bass_kernel_spmd(nc, in_maps, core_ids=list(range(NCORES)), **kw)
    out = np.empty((n_dec, B, N), np.float32)
    for c in range(NCORES):
        out[:, c * BL : (c + 1) * BL, :] = (
            res.results[c]["outs"].astype(np.float32).reshape(n_dec, BL, N)
        )
    return out.reshape(n_dec, B, N), res


def kernel(**inputs) -> np.ndarray:
    out, _ = _run(inputs)
    return out.reshape(HOR, B, N)


# revision 46
# speedup vs baseline: 1.0151x; 1.0151x over previous
"""DCRNN (2-layer encoder/decoder DCGRU, N=512 nodes, B=32, U=64, K=2, 2 supports)
Trainium2 Bass/Tile kernel, data-parallel over batch across 8 NeuronCores.

Key reformulation: the Chebyshev graph conv
    gconv(X) = sum_m T_m(S) @ X @ W_m,  T in {I, S_A, 2S_A^2-I, S_B, 2S_B^2-I}
with the T_m precomputed on host. Per gconv:
  stage 1 (dense):     A_m = X @ W_m      -- activations kept channel-on-partition
                       (X^T used as matmul lhsT, W as rhs -> A node-on-partition)
  stage 2 (diffusion): out^T = X @ W_0 + sum_{m>0} (T_m A_m)^T -- A_m as lhsT,
                       T_m^T as rhs, accumulated over m in PSUM; bias+sigmoid/
                       tanh fused into the PSUM->SBUF move.
This needs no tensor transposes at runtime. All weights/supports/state resident
in SBUF for the whole 24-step recurrence; only encoder inputs in / decoder
outputs out touch HBM.

All matmul operands are bf16 (PSUM accumulation stays fp32): fp32r stationary
operands disable the PE's fast-weight-load path, serializing an LDWEIGHTS
(~220ns) with every MATMUL; bf16 streams at the same 1 cycle/row but lets the
weight load overlap. bf16 state also doubles DVE elementwise throughput.

Scheduling notes (from HW traces): engine queues are strictly in-order, so
the program order must avoid head-of-line blocking -- stage-1 for all four
local batches runs before any stage-2 so the PSUM->SBUF drains have matmul
cover; the decoder projection runs after BOTH layer-1 candidate phases so its
wait on the GRU tail doesn't block ready matmuls; layer-0's new hidden state
is written directly into the layer-1 input tile (its immediate consumer) with
the other copies pushed off the critical path onto gpsimd.
"""

import sys

sys.path.insert(0, "/opt/trn_rl_repo")

import numpy as np

import concourse.bass as bass
import concourse.mybir as mybir
import concourse.tile as tile
from concourse import bacc, bass_utils

# Model dims (fixed by the problem)
N = 512
T_ENC = 12
HOR = 12
U = 64
NM = 5  # num diffusion matrices (I + 2 per support * 2 supports)
B = 32
NCORES = 8
BL = B // NCORES  # local batch = 4
BI = BL * N  # 2048: the (b, node) free dim
C0 = 1 + U  # 65 input channels, layer 0
C1 = U + U  # 128 input channels, layer 1
KCH = N // 128  # 4 node chunks

F32 = mybir.dt.float32
BF16 = mybir.dt.bfloat16
AF = mybir.ActivationFunctionType


def _build_program(n_enc=T_ENC, n_dec=HOR):
    nc = bacc.Bacc("TRN2", target_bir_lowering=False, debug=False)

    # ---- DRAM I/O ----
    d_xenc = nc.dram_tensor("xenc", [n_enc, BI], BF16, kind="ExternalInput")
    # diffusion matrices m=1..4 (identity term is folded into stage 2)
    d_tm = nc.dram_tensor("tmats", [(NM - 1) * KCH * 128, 512], BF16,
                          kind="ExternalInput")
    d_w = {}
    for pfx in ("e", "d"):
        for lyr, c_in in ((0, C0), (1, C1)):
            d_w[f"{pfx}wg{lyr}"] = nc.dram_tensor(
                f"{pfx}wg{lyr}", [c_in, NM * 2 * U], BF16, kind="ExternalInput"
            )
            d_w[f"{pfx}wc{lyr}"] = nc.dram_tensor(
                f"{pfx}wc{lyr}", [c_in, NM * U], BF16, kind="ExternalInput"
            )
            d_w[f"{pfx}bgru{lyr}"] = nc.dram_tensor(
                f"{pfx}bgru{lyr}", [2 * U, 1], F32, kind="ExternalInput"
            )
            d_w[f"{pfx}bc{lyr}"] = nc.dram_tensor(
                f"{pfx}bc{lyr}", [U, 1], F32, kind="ExternalInput"
            )
    d_pw = nc.dram_tensor("pw", [U, 2], BF16, kind="ExternalInput")
    d_pb = nc.dram_tensor("pb", [1, 1], F32, kind="ExternalInput")
    d_out = nc.dram_tensor("outs", [n_dec, BI], BF16, kind="ExternalOutput")

    with tile.TileContext(nc) as tc:
        _body(tc, n_enc, n_dec, d_xenc, d_tm, d_w, d_pw, d_pb, d_out)
    nc.compile()
    return nc


def _body(tc, n_enc, n_dec, d_xenc, d_tm, d_w, d_pw, d_pb, d_out):
    nc = tc.nc
    consts = tc.alloc_tile_pool(name="consts", bufs=1)
    work = tc.alloc_tile_pool(name="work", bufs=1)
    gpool = tc.alloc_tile_pool(name="gpool", bufs=2)
    # >=16 ag / >=8 ac bufs so a full cell's stage-1 tiles are live at once
    # (fewer forces stage-1 drains to WAR-wait on stage-2's reads)
    ag_pool = tc.alloc_tile_pool(name="agp", bufs=18)
    ac_pool = tc.alloc_tile_pool(name="acp", bufs=10)
    # 6 stage-1 PSUM banks let the 16-matmul stage-1 burst run ~6 deep ahead
    # of the scalar/vector drains (687/551ns each vs 216ns matmul issue); the
    # drains catch up during the drain-free stage-2 window. stage-2 accs
    # only need 2 in flight (acts on acc N overlap matmuls of acc N+1).
    ps1 = tc.alloc_tile_pool(name="ps1", bufs=6, space="PSUM")
    ps2 = tc.alloc_tile_pool(name="ps2", bufs=2, space="PSUM")

    # ---- resident constants ----
    # one tile per (m, k) chunk so early stage-2 consumers wait only on their
    # own chunk's DMA, not the whole 2.1 MB load (DMAs issued after the
    # barrier below so the first gate matmuls don't wait for them)
    tm_sb = {}
    for m in range(1, NM):
        for k in range(KCH):
            tm_sb[(m, k)] = consts.tile([128, 512], BF16, name=f"tm_{m}_{k}")

    w_sb = {}
    for key, dt in d_w.items():
        w_sb[key] = consts.tile(list(dt.shape), dt.dtype, name=f"sb_{key}")
    pw_sb = consts.tile([U, 2], BF16, name="pw_sb")
    pb_sb = consts.tile([1, 1], F32, name="pb_sb")

    def load_weights(pfx):
        for key, dt in d_w.items():
            if key[0] == pfx:
                nc.sync.dma_start(out=w_sb[key][:, :], in_=dt[:, :])

    load_weights("e")

    # ---- persistent state (channel-on-partition, free dim = (b, node)) ----
    X0 = work.tile([C0, BI], BF16, name="X0")  # [h0 ; x]
    X0c = work.tile([C0, BI], BF16, name="X0c")  # [r*h0 ; x]
    X1 = work.tile([C1, BI], BF16, name="X1")  # [h0 ; h1]
    X1c = work.tile([C1, BI], BF16, name="X1c")  # [r*h1 ; h0]
    h1t = work.tile([U, BI], BF16, name="h1t")  # h1 at partition base 0

    nc.vector.memset(X0[0:U, :], 0.0)
    nc.vector.memset(X1c[U:C1, :], 0.0)
    nc.gpsimd.memset(X1[U:C1, :], 0.0)
    nc.vector.memset(h1t[:, :], 0.0)

    # collapse the enc-weight/init dependencies into one semaphore so the
    # first consumers don't exceed per-instruction sync-wait slots; decoder
    # weights and the tm chunks ride their own DMA deps
    tc.strict_bb_all_engine_barrier()

    for m in range(1, NM):
        for k in range(KCH):
            row = ((m - 1) * KCH + k) * 128
            # ride the idle gpsimd queue: its DMA completion counter is
            # separate from the sync queue's, so the first gate matmuls
            # (waiting on enc weights via sync) don't also wait these 2.1 MB
            nc.gpsimd.dma_start(out=tm_sb[(m, k)][:, :], in_=d_tm[row : row + 128, :])

    def cell_phases(X, Xc, c_in, rh_lo, h_src, h64_src, wg, bgru, wc, bc,
                    h_writer, post):
        """One DCGRU cell split into stage-1/stage-2 phases, each over
        batch-pair p in {0, 1}."""
        # gate outputs in one [u ; r] tile (weight columns permuted on host):
        # u at base 0 pairs with h_src (base 0); r at base 64 pairs with the
        # resident base-64 h copy (TensorTensor needs equal input bases)
        RU = gpool.tile([2 * U, BI], BF16, tag="RU", name="RU", bufs=1)
        Um1 = gpool.tile([U, BI], BF16, tag="Um1", name="Um1", bufs=1)  # u - 1
        uh = gpool.tile([U, BI], BF16, tag="uh", name="uh", bufs=1)
        Ct = gpool.tile([U, BI], BF16, tag="Ct", name="Ct", bufs=1)
        wc_t = gpool.tile([U, BI], BF16, tag="wct", name="wc_t", bufs=1)
        ag = {}
        ac = {}

        def gate_s1(p):
            # stage 1: A_m = X @ Wg_m for m=1..4 (m=0 folded into stage 2)
            for b in (2 * p, 2 * p + 1):
                for k in range(KCH):
                    pg = ps1.tile([128, 512], F32, tag="s1", name="pg")
                    lhsT = X[0:c_in, b * N + k * 128 : b * N + (k + 1) * 128]
                    nc.tensor.matmul(
                        pg, lhsT, wg[:, 128:640], start=True, stop=True
                    )
                    a = ag_pool.tile([128, 4 * 128], BF16, tag="ag", name="ag")
                    ag[(b, k)] = a
                    # split PSUM->SBUF drains between scalar and vector so
                    # neither engine serializes behind the whole batch
                    if k % 2 == 0:
                        nc.scalar.copy(out=a[:, :], in_=pg)
                    else:
                        nc.vector.tensor_copy(out=a[:, :], in_=pg)

        def gate_s2(p):
            # stage 2: acc = X @ Wg_0 + sum_{m>0} (T_m A_m)^T, fused sigmoid
            for b in (2 * p, 2 * p + 1):
                acc = ps2.tile([128, 512], F32, tag="s2", name="accg")
                nc.tensor.matmul(
                    acc,
                    wg[:, 0:128],
                    X[0:c_in, b * N : (b + 1) * N],
                    start=True,
                    stop=False,
                )
                for m in range(1, NM):
                    for k in range(KCH):
                        nc.tensor.matmul(
                            acc,
                            ag[(b, k)][:, (m - 1) * 128 : m * 128],
                            tm_sb[(m, k)],
                            start=False,
                            stop=(m == NM - 1 and k == KCH - 1),
                        )
                bcols = slice(b * N, (b + 1) * N)
                nc.scalar.activation(
                    out=RU[:, bcols], in_=acc[0 : 2 * U, :], func=AF.Sigmoid,
                    bias=bgru[:, 0:1], scale=1.0,
                )
            pcols = slice(2 * p * N, 2 * (p + 1) * N)
            # r*h -> candidate input rows; u*h for the GRU blend; (u-1) for
            # the blend's candidate half, computed off the critical tail
            nc.vector.tensor_mul(
                out=Xc[rh_lo : rh_lo + U, pcols],
                in0=RU[U : 2 * U, pcols],
                in1=h64_src[:, pcols],
            )
            nc.vector.tensor_mul(
                out=uh[:, pcols],
                in0=RU[0:U, pcols],
                in1=h_src[:, pcols],
            )
            nc.vector.tensor_scalar_sub(Um1[:, pcols], RU[0:U, pcols], 1.0)

        def cand_s1(p):
            # stage 1: A_m^c = Xc @ Wc_m for m=1..4 (m=0 folded into stage 2)
            for b in (2 * p, 2 * p + 1):
                for k in range(KCH):
                    pc = ps1.tile([128, 512], F32, tag="s1", name="pc")
                    lhsT = Xc[0:c_in, b * N + k * 128 : b * N + (k + 1) * 128]
                    nc.tensor.matmul(
                        pc[:, 0:256], lhsT, wc[:, U:], start=True, stop=True
                    )
                    if b % 2 == 0:
                        a = ac_pool.tile([128, NM - 1, 2, U], BF16, tag="ac", name="ac")
                        ac[(p, k)] = a
                    dst = ac[(p, k)][:, :, b % 2, :]
                    src_v = pc[:, 0:256].rearrange("p (m u) -> p m u", m=NM - 1)
                    if k % 2 == 0:
                        nc.scalar.copy(out=dst, in_=src_v)
                    else:
                        nc.vector.tensor_copy(out=dst, in_=src_v)

        def cand_s2(p):
            acc = ps2.tile([128, 512], F32, tag="s2", name="accc")
            # m=0 (identity diffusion) comes straight from Xc @ Wc_0, one
            # 64-partition half of the accumulator per batch
            for half in range(2):
                b = 2 * p + half
                nc.tensor.matmul(
                    acc[half * U : (half + 1) * U, :],
                    wc[:, 0:U],
                    Xc[0:c_in, b * N : (b + 1) * N],
                    start=True,
                    stop=False,
                    skip_group_check=True,
                )
            for m in range(1, NM):
                for k in range(KCH):
                    nc.tensor.matmul(
                        acc,
                        ac[(p, k)][:, m - 1, :, :],
                        tm_sb[(m, k)],
                        start=False,
                        stop=(m == NM - 1 and k == KCH - 1),
                        skip_group_check=True,
                    )
            for half in range(2):
                b = 2 * p + half
                bcols = slice(b * N, (b + 1) * N)
                nc.scalar.activation(
                    out=Ct[:, bcols],
                    in_=acc[half * U : (half + 1) * U, :],
                    func=AF.Tanh,
                    bias=bc[:, 0:1],
                    scale=1.0,
                )
            pcols = slice(2 * p * N, 2 * (p + 1) * N)
            # h_new = u*h + (1-u)*c == u*h - (u-1)*c
            nc.vector.tensor_mul(out=wc_t[:, pcols], in0=Um1[:, pcols], in1=Ct[:, pcols])
            nc.vector.tensor_sub(
                out=h_writer(p, pcols), in0=uh[:, pcols], in1=wc_t[:, pcols]
            )
            post(p, pcols)

        return gate_s1, gate_s2, cand_s1, cand_s2

    def l0_writer(p, pcols):
        # write h0 straight into its immediate consumer (layer-1 gate lhsT);
        # the copies for next step / candidate input happen off the critical
        # path on gpsimd
        return X1[0:U, pcols]

    def _copy(out, in_):
        # same-dtype COPY (and anything on gpsimd) runs several times slower
        # than the DVE tensor-scalar path; express state copies as +0.0 on
        # vector
        nc.vector.tensor_scalar_add(out, in_, 0.0)

    def l0_post(p, pcols):
        _copy(X1c[U:C1, pcols], X1[0:U, pcols])
        _copy(X0[0:U, pcols], X1[0:U, pcols])

    def l1_writer(p, pcols):
        return h1t[:, pcols]

    def l1_post(p, pcols):
        _copy(X1[U:C1, pcols], h1t[:, pcols])

    def proj_phase(p):
        # projection for pair p: out = h1 . pw + pb -> feeds back as x row.
        # pp tiles come from ps1 (idle during the projection) so both
        # quarters' matmuls can be in flight; the X0-row acts go first since
        # the next step's gate m=0 matmuls consume them before X0c is needed.
        pps = {}
        for q in (2 * p, 2 * p + 1):
            pp = ps1.tile([2, 512], F32, tag="s1", name="pp")
            nc.tensor.matmul(
                pp,
                pw_sb[:, 0:2],
                h1t[:, q * 512 : (q + 1) * 512],
                start=True,
                stop=True,
            )
            pps[q] = pp
        for q in (2 * p, 2 * p + 1):
            nc.scalar.activation(
                out=X0[U:C0, q * 512 : (q + 1) * 512],
                in_=pps[q][0:1, :],
                func=AF.Identity,
                bias=pb_sb[:, 0:1],
                scale=1.0,
            )
        for q in (2 * p, 2 * p + 1):
            nc.scalar.activation(
                out=X0c[U:C0, q * 512 : (q + 1) * 512],
                in_=pps[q][0:1, :],
                func=AF.Identity,
                bias=pb_sb[:, 0:1],
                scale=1.0,
            )

    def run_step(pfx, dec_t=None):
        gs1_0, gs2_0, cs1_0, cs2_0 = cell_phases(
            X0, X0c, C0, 0, X0[0:U, :], X1c[U:C1, :],
            w_sb[f"{pfx}wg0"], w_sb[f"{pfx}bgru0"],
            w_sb[f"{pfx}wc0"], w_sb[f"{pfx}bc0"],
            l0_writer, l0_post,
        )
        gs1_1, gs2_1, cs1_1, cs2_1 = cell_phases(
            X1, X1c, C1, 0, h1t[:, :], X1[U:C1, :],
            w_sb[f"{pfx}wg1"], w_sb[f"{pfx}bgru1"],
            w_sb[f"{pfx}wc1"], w_sb[f"{pfx}bc1"],
            l1_writer, l1_post,
        )
        gs1_0(0); gs1_0(1); gs2_0(0); gs2_0(1)
        cs1_0(0); cs1_0(1); cs2_0(0); cs2_0(1)
        gs1_1(0); gs1_1(1); gs2_1(0); gs2_1(1)
        cs1_1(0); cs1_1(1); cs2_1(0); cs2_1(1)
        if dec_t is not None:
            proj_phase(0)
            proj_phase(1)
            nc.sync.dma_start(out=d_out[dec_t : dec_t + 1, :], in_=X0[U:C0, :])

    # ================= encoder =================
    for t in range(n_enc):
        nc.sync.dma_start(out=X0[U:C0, :], in_=d_xenc[t : t + 1, :])
        nc.sync.dma_start(out=X0c[U:C0, :], in_=d_xenc[t : t + 1, :])
        run_step("e")
        if t == 0:
            # stream decoder weights in during encoder compute; the sync
            # queue is idle from here until the next step's input DMA
            load_weights("d")
            nc.sync.dma_start(out=pw_sb[:, :], in_=d_pw[:, :])
            nc.sync.dma_start(out=pb_sb, in_=d_pb[:, :])

    # ================= decoder =================
    nc.gpsimd.memset(X0[U:C0, :], 0.0)
    nc.gpsimd.memset(X0c[U:C0, :], 0.0)
    for t in range(n_dec):
        run_step("d", dec_t=t)

    for pool in (ps2, ps1, ac_pool, ag_pool, gpool, work, consts):
        pool.release()


# --------------------------------------------------------------------------
# host-side packing
# --------------------------------------------------------------------------
def _prep_shared(inputs):
    import ml_dtypes

    bf16 = ml_dtypes.bfloat16
    sup = np.asarray(inputs["supports"], np.float64)
    eye = np.eye(N, dtype=np.float64)
    tms = [
        sup[0],
        2.0 * (sup[0] @ sup[0]) - eye,
        sup[1],
        2.0 * (sup[1] @ sup[1]) - eye,
    ]
    tmats = np.stack([t.T for t in tms]).astype(np.float32)  # [m-1, j, i]
    tmats = tmats.reshape((NM - 1) * KCH * 128, 512)

    shared = {"tmats": np.ascontiguousarray(tmats.astype(bf16))}
    for pfx, name in (("e", "enc"), ("d", "dec")):
        for lyr, c_in in ((0, C0), (1, C1)):
            wg = np.asarray(inputs[f"{name}{lyr}_Wg"], np.float32)
            wc = np.asarray(inputs[f"{name}{lyr}_Wc"], np.float32)
            wg = wg.reshape(c_in, NM * 2 * U)
            wc = wc.reshape(c_in, NM * U)
            bg = np.asarray(inputs[f"{name}{lyr}_bg"], np.float32)
            bc = np.asarray(inputs[f"{name}{lyr}_bc"], np.float32)
            perm_ur = np.r_[U : 2 * U, 0:U]  # gate out-channels as [u ; r]
            wg_r = wg.reshape(c_in, NM, 2 * U)[:, :, perm_ur].reshape(
                c_in, NM * 2 * U
            )
            wc_r = wc.reshape(c_in, NM, U).reshape(c_in, NM * U)
            if lyr == 0:
                perm = np.r_[1:c_in, 0]  # rows [h..., x]
                wg_r = wg_r[perm]
                wc_r = wc_r[perm]
            else:
                # X1c rows are [r*h1 ; h0]: candidate weight rows follow
                wc_r = wc_r[np.r_[U:c_in, 0:U]]
            shared[f"{pfx}wg{lyr}"] = np.ascontiguousarray(wg_r.astype(bf16))
            shared[f"{pfx}wc{lyr}"] = np.ascontiguousarray(wc_r.astype(bf16))
            shared[f"{pfx}bgru{lyr}"] = np.ascontiguousarray(
                np.concatenate([bg[U:], bg[:U]]).reshape(2 * U, 1)
            )
            shared[f"{pfx}bc{lyr}"] = np.ascontiguousarray(bc.reshape(U, 1))
    pw = np.asarray(inputs["proj_W"], np.float32).reshape(U, 1)
    shared["pw"] = np.ascontiguousarray(
        np.concatenate([pw, np.zeros((U, 1), np.float32)], axis=1).astype(bf16)
    )
    shared["pb"] = np.asarray(inputs["proj_b"], np.float32).reshape(1, 1)
    return shared


def _make_in_maps(inputs, n_enc=T_ENC):
    import ml_dtypes

    shared = _prep_shared(inputs)
    x = np.asarray(inputs["inputs"], np.float32)  # (T, B, N)
    in_maps = []
    for c in range(NCORES):
        m = dict(shared)
        m["xenc"] = np.ascontiguousarray(
            x[:n_enc, c * BL : (c + 1) * BL, :]
            .reshape(n_enc, BI)
            .astype(ml_dtypes.bfloat16)
        )
        in_maps.append(m)
    return in_maps


_PROG_CACHE = {}


def _get_program(n_enc=T_ENC, n_dec=HOR):
    key = (n_enc, n_dec)
    if key not in _PROG_CACHE:
        _PROG_CACHE[key] = _build_program(n_enc, n_dec)
    return _PROG_CACHE[key]


def _run(inputs, n_enc=T_ENC, n_dec=HOR, **kw):
    nc = _get_program(n_enc, n_dec)
    in_maps = _make_in_maps(inputs, n_enc)
    res = bass_utils.run_bass_kernel_spmd(nc, in_maps, core_ids=list(range(NCORES)), **kw)
    out = np.empty((n_dec, B, N), np.float32)
    for c in range(NCORES):
        out[:, c * BL : (c + 1) * BL, :] = (
            res.results[c]["outs"].astype(np.float32).reshape(n_dec, BL, N)
        )
    return out.reshape(n_dec, B, N), res


def kernel(**inputs) -> np.ndarray:
    out, _ = _run(inputs)
    return out.reshape(HOR, B, N)


# revision 47
# speedup vs baseline: 1.0173x; 1.0021x over previous
"""DCRNN (2-layer encoder/decoder DCGRU, N=512 nodes, B=32, U=64, K=2, 2 supports)
Trainium2 Bass/Tile kernel, data-parallel over batch across 8 NeuronCores.

Key reformulation: the Chebyshev graph conv
    gconv(X) = sum_m T_m(S) @ X @ W_m,  T in {I, S_A, 2S_A^2-I, S_B, 2S_B^2-I}
with the T_m precomputed on host. Per gconv:
  stage 1 (dense):     A_m = X @ W_m      -- activations kept channel-on-partition
                       (X^T used as matmul lhsT, W as rhs -> A node-on-partition)
  stage 2 (diffusion): out^T = X @ W_0 + sum_{m>0} (T_m A_m)^T -- A_m as lhsT,
                       T_m^T as rhs, accumulated over m in PSUM; bias+sigmoid/
                       tanh fused into the PSUM->SBUF move.
This needs no tensor transposes at runtime. All weights/supports/state resident
in SBUF for the whole 24-step recurrence; only encoder inputs in / decoder
outputs out touch HBM.

All matmul operands are bf16 (PSUM accumulation stays fp32): fp32r stationary
operands disable the PE's fast-weight-load path, serializing an LDWEIGHTS
(~220ns) with every MATMUL; bf16 streams at the same 1 cycle/row but lets the
weight load overlap. bf16 state also doubles DVE elementwise throughput.

Scheduling notes (from HW traces): engine queues are strictly in-order, so
the program order must avoid head-of-line blocking -- stage-1 for all four
local batches runs before any stage-2 so the PSUM->SBUF drains have matmul
cover; the decoder projection runs after BOTH layer-1 candidate phases so its
wait on the GRU tail doesn't block ready matmuls; layer-0's new hidden state
is written directly into the layer-1 input tile (its immediate consumer) with
the other copies pushed off the critical path onto gpsimd.
"""

import sys

sys.path.insert(0, "/opt/trn_rl_repo")

import numpy as np

import concourse.bass as bass
import concourse.mybir as mybir
import concourse.tile as tile
from concourse import bacc, bass_utils

# Model dims (fixed by the problem)
N = 512
T_ENC = 12
HOR = 12
U = 64
NM = 5  # num diffusion matrices (I + 2 per support * 2 supports)
B = 32
NCORES = 8
BL = B // NCORES  # local batch = 4
BI = BL * N  # 2048: the (b, node) free dim
C0 = 1 + U  # 65 input channels, layer 0
C1 = U + U  # 128 input channels, layer 1
KCH = N // 128  # 4 node chunks

F32 = mybir.dt.float32
BF16 = mybir.dt.bfloat16
AF = mybir.ActivationFunctionType


def _build_program(n_enc=T_ENC, n_dec=HOR):
    nc = bacc.Bacc("TRN2", target_bir_lowering=False, debug=False)

    # ---- DRAM I/O ----
    d_xenc = nc.dram_tensor("xenc", [n_enc, BI], BF16, kind="ExternalInput")
    # diffusion matrices m=1..4 (identity term is folded into stage 2)
    d_tm = nc.dram_tensor("tmats", [(NM - 1) * KCH * 128, 512], BF16,
                          kind="ExternalInput")
    d_w = {}
    for pfx in ("e", "d"):
        for lyr, c_in in ((0, C0), (1, C1)):
            d_w[f"{pfx}wg{lyr}"] = nc.dram_tensor(
                f"{pfx}wg{lyr}", [c_in, NM * 2 * U], BF16, kind="ExternalInput"
            )
            d_w[f"{pfx}wc{lyr}"] = nc.dram_tensor(
                f"{pfx}wc{lyr}", [c_in, NM * U], BF16, kind="ExternalInput"
            )
            d_w[f"{pfx}bgru{lyr}"] = nc.dram_tensor(
                f"{pfx}bgru{lyr}", [2 * U, 1], F32, kind="ExternalInput"
            )
            d_w[f"{pfx}bc{lyr}"] = nc.dram_tensor(
                f"{pfx}bc{lyr}", [U, 1], F32, kind="ExternalInput"
            )
    d_pw = nc.dram_tensor("pw", [U, 2], BF16, kind="ExternalInput")
    d_pb = nc.dram_tensor("pb", [1, 1], F32, kind="ExternalInput")
    d_out = nc.dram_tensor("outs", [n_dec, BI], BF16, kind="ExternalOutput")

    with tile.TileContext(nc) as tc:
        _body(tc, n_enc, n_dec, d_xenc, d_tm, d_w, d_pw, d_pb, d_out)
    nc.compile()
    return nc


def _body(tc, n_enc, n_dec, d_xenc, d_tm, d_w, d_pw, d_pb, d_out):
    nc = tc.nc
    consts = tc.alloc_tile_pool(name="consts", bufs=1)
    work = tc.alloc_tile_pool(name="work", bufs=1)
    gpool = tc.alloc_tile_pool(name="gpool", bufs=2)
    # >=16 ag / >=8 ac bufs so a full cell's stage-1 tiles are live at once
    # (fewer forces stage-1 drains to WAR-wait on stage-2's reads)
    ag_pool = tc.alloc_tile_pool(name="agp", bufs=18)
    ac_pool = tc.alloc_tile_pool(name="acp", bufs=10)
    # 6 stage-1 PSUM banks let the 16-matmul stage-1 burst run ~6 deep ahead
    # of the scalar/vector drains (687/551ns each vs 216ns matmul issue); the
    # drains catch up during the drain-free stage-2 window. stage-2 accs
    # only need 2 in flight (acts on acc N overlap matmuls of acc N+1).
    ps1 = tc.alloc_tile_pool(name="ps1", bufs=6, space="PSUM")
    ps2 = tc.alloc_tile_pool(name="ps2", bufs=2, space="PSUM")

    # ---- resident constants ----
    # one tile per (m, k) chunk so early stage-2 consumers wait only on their
    # own chunk's DMA, not the whole 2.1 MB load (DMAs issued after the
    # barrier below so the first gate matmuls don't wait for them)
    tm_sb = {}
    for m in range(1, NM):
        for k in range(KCH):
            tm_sb[(m, k)] = consts.tile([128, 512], BF16, name=f"tm_{m}_{k}")

    w_sb = {}
    for key, dt in d_w.items():
        w_sb[key] = consts.tile(list(dt.shape), dt.dtype, name=f"sb_{key}")
    pw_sb = consts.tile([U, 2], BF16, name="pw_sb")
    pb_sb = consts.tile([1, 1], F32, name="pb_sb")

    def load_weights(pfx):
        for key, dt in d_w.items():
            if key[0] == pfx:
                nc.sync.dma_start(out=w_sb[key][:, :], in_=dt[:, :])

    load_weights("e")

    # ---- persistent state (channel-on-partition, free dim = (b, node)) ----
    X0 = work.tile([C0, BI], BF16, name="X0")  # [h0 ; x]
    X0c = work.tile([C0, BI], BF16, name="X0c")  # [r*h0 ; x]
    X1 = work.tile([C1, BI], BF16, name="X1")  # [h0 ; h1]
    X1c = work.tile([C1, BI], BF16, name="X1c")  # [r*h1 ; h0]
    h1t = work.tile([U, BI], BF16, name="h1t")  # h1 at partition base 0

    nc.vector.memset(X0[0:U, :], 0.0)
    nc.vector.memset(X1c[U:C1, :], 0.0)
    nc.gpsimd.memset(X1[U:C1, :], 0.0)
    nc.vector.memset(h1t[:, :], 0.0)

    # collapse the enc-weight/init dependencies into one semaphore so the
    # first consumers don't exceed per-instruction sync-wait slots; decoder
    # weights and the tm chunks ride their own DMA deps
    tc.strict_bb_all_engine_barrier()

    for m in range(1, NM):
        for k in range(KCH):
            row = ((m - 1) * KCH + k) * 128
            # ride the idle gpsimd queue: its DMA completion counter is
            # separate from the sync queue's, so the first gate matmuls
            # (waiting on enc weights via sync) don't also wait these 2.1 MB
            nc.gpsimd.dma_start(out=tm_sb[(m, k)][:, :], in_=d_tm[row : row + 128, :])

    def cell_phases(X, Xc, c_in, rh_lo, h_src, h64_src, wg, bgru, wc, bc,
                    h_writer, post):
        """One DCGRU cell split into stage-1/stage-2 phases, each over
        batch-pair p in {0, 1}."""
        # gate outputs in one [u ; r] tile (weight columns permuted on host):
        # u at base 0 pairs with h_src (base 0); r at base 64 pairs with the
        # resident base-64 h copy (TensorTensor needs equal input bases)
        RU = gpool.tile([2 * U, BI], BF16, tag="RU", name="RU", bufs=1)
        Um1 = gpool.tile([U, BI], BF16, tag="Um1", name="Um1", bufs=1)  # u - 1
        uh = gpool.tile([U, BI], BF16, tag="uh", name="uh", bufs=1)
        Ct = gpool.tile([U, BI], BF16, tag="Ct", name="Ct", bufs=1)
        wc_t = gpool.tile([U, BI], BF16, tag="wct", name="wc_t", bufs=1)
        ag = {}
        ac = {}

        def gate_s1(p):
            # stage 1: A_m = X @ Wg_m for m=1..4 (m=0 folded into stage 2)
            for b in (2 * p, 2 * p + 1):
                for k in range(KCH):
                    pg = ps1.tile([128, 512], F32, tag="s1", name="pg")
                    lhsT = X[0:c_in, b * N + k * 128 : b * N + (k + 1) * 128]
                    nc.tensor.matmul(
                        pg, lhsT, wg[:, 128:640], start=True, stop=True
                    )
                    a = ag_pool.tile([128, 4 * 128], BF16, tag="ag", name="ag")
                    ag[(b, k)] = a
                    # split PSUM->SBUF drains between scalar and vector so
                    # neither engine serializes behind the whole batch
                    if k % 2 == 0:
                        nc.scalar.copy(out=a[:, :], in_=pg)
                    else:
                        nc.vector.tensor_copy(out=a[:, :], in_=pg)

        def gate_s2(p):
            # stage 2: acc = X @ Wg_0 + sum_{m>0} (T_m A_m)^T, fused sigmoid
            for b in (2 * p, 2 * p + 1):
                acc = ps2.tile([128, 512], F32, tag="s2", name="accg")
                nc.tensor.matmul(
                    acc,
                    wg[:, 0:128],
                    X[0:c_in, b * N : (b + 1) * N],
                    start=True,
                    stop=False,
                )
                for m in range(1, NM):
                    for k in range(KCH):
                        nc.tensor.matmul(
                            acc,
                            ag[(b, k)][:, (m - 1) * 128 : m * 128],
                            tm_sb[(m, k)],
                            start=False,
                            stop=(m == NM - 1 and k == KCH - 1),
                        )
                bcols = slice(b * N, (b + 1) * N)
                nc.scalar.activation(
                    out=RU[:, bcols], in_=acc[0 : 2 * U, :], func=AF.Sigmoid,
                    bias=bgru[:, 0:1], scale=1.0,
                )
            pcols = slice(2 * p * N, 2 * (p + 1) * N)
            # r*h -> candidate input rows; u*h for the GRU blend; (u-1) for
            # the blend's candidate half, computed off the critical tail
            nc.vector.tensor_mul(
                out=Xc[rh_lo : rh_lo + U, pcols],
                in0=RU[U : 2 * U, pcols],
                in1=h64_src[:, pcols],
            )
            nc.vector.tensor_mul(
                out=uh[:, pcols],
                in0=RU[0:U, pcols],
                in1=h_src[:, pcols],
            )
            nc.vector.tensor_scalar_sub(Um1[:, pcols], RU[0:U, pcols], 1.0)

        def cand_s1(p):
            # stage 1: A_m^c = Xc @ Wc_m for m=1..4 (m=0 folded into stage 2).
            # Each [128,256] result only half-fills a PSUM bank, so pack two
            # k-chunks per bank and drain both with one instruction -- halves
            # the drain count and the bank-recycle pressure that paces this
            # burst (110ns matmul issue vs ~400ns drains).
            for b in (2 * p, 2 * p + 1):
                for kk in range(2):
                    pc = ps1.tile([128, 512], F32, tag="s1", name="pc")
                    for h in range(2):
                        k = 2 * kk + h
                        lhsT = Xc[0:c_in, b * N + k * 128 : b * N + (k + 1) * 128]
                        nc.tensor.matmul(
                            pc[:, h * 256 : (h + 1) * 256],
                            lhsT,
                            wc[:, U:],
                            start=True,
                            stop=True,
                            skip_group_check=True,
                        )
                    if b % 2 == 0:
                        a = ac_pool.tile(
                            [128, 2, NM - 1, 2, U], BF16, tag="ac", name="ac"
                        )
                        ac[(p, kk)] = a
                    dst = ac[(p, kk)][:, :, :, b % 2, :]
                    src_v = pc[:, :].rearrange(
                        "p (kk m u) -> p kk m u", kk=2, m=NM - 1
                    )
                    if kk % 2 == 0:
                        nc.scalar.copy(out=dst, in_=src_v)
                    else:
                        nc.vector.tensor_copy(out=dst, in_=src_v)

        def cand_s2(p):
            acc = ps2.tile([128, 512], F32, tag="s2", name="accc")
            # m=0 (identity diffusion) comes straight from Xc @ Wc_0, one
            # 64-partition half of the accumulator per batch
            for half in range(2):
                b = 2 * p + half
                nc.tensor.matmul(
                    acc[half * U : (half + 1) * U, :],
                    wc[:, 0:U],
                    Xc[0:c_in, b * N : (b + 1) * N],
                    start=True,
                    stop=False,
                    skip_group_check=True,
                )
            for m in range(1, NM):
                for k in range(KCH):
                    nc.tensor.matmul(
                        acc,
                        ac[(p, k // 2)][:, k % 2, m - 1, :, :],
                        tm_sb[(m, k)],
                        start=False,
                        stop=(m == NM - 1 and k == KCH - 1),
                        skip_group_check=True,
                    )
            for half in range(2):
                b = 2 * p + half
                bcols = slice(b * N, (b + 1) * N)
                nc.scalar.activation(
                    out=Ct[:, bcols],
                    in_=acc[half * U : (half + 1) * U, :],
                    func=AF.Tanh,
                    bias=bc[:, 0:1],
                    scale=1.0,
                )
            pcols = slice(2 * p * N, 2 * (p + 1) * N)
            # h_new = u*h + (1-u)*c == u*h - (u-1)*c
            nc.vector.tensor_mul(out=wc_t[:, pcols], in0=Um1[:, pcols], in1=Ct[:, pcols])
            nc.vector.tensor_sub(
                out=h_writer(p, pcols), in0=uh[:, pcols], in1=wc_t[:, pcols]
            )
            post(p, pcols)

        return gate_s1, gate_s2, cand_s1, cand_s2

    def l0_writer(p, pcols):
        # write h0 straight into its immediate consumer (layer-1 gate lhsT);
        # the copies for next step / candidate input happen off the critical
        # path on gpsimd
        return X1[0:U, pcols]

    def _copy(out, in_):
        # same-dtype COPY (and anything on gpsimd) runs several times slower
        # than the DVE tensor-scalar path; express state copies as +0.0 on
        # vector
        nc.vector.tensor_scalar_add(out, in_, 0.0)

    def l0_post(p, pcols):
        _copy(X1c[U:C1, pcols], X1[0:U, pcols])
        _copy(X0[0:U, pcols], X1[0:U, pcols])

    def l1_writer(p, pcols):
        return h1t[:, pcols]

    def l1_post(p, pcols):
        _copy(X1[U:C1, pcols], h1t[:, pcols])

    def proj_phase(p):
        # projection for pair p: out = h1 . pw + pb -> feeds back as x row.
        # pp tiles come from ps1 (idle during the projection) so both
        # quarters' matmuls can be in flight; the X0-row acts go first since
        # the next step's gate m=0 matmuls consume them before X0c is needed.
        pps = {}
        for q in (2 * p, 2 * p + 1):
            pp = ps1.tile([2, 512], F32, tag="s1", name="pp")
            nc.tensor.matmul(
                pp,
                pw_sb[:, 0:2],
                h1t[:, q * 512 : (q + 1) * 512],
                start=True,
                stop=True,
            )
            pps[q] = pp
        for q in (2 * p, 2 * p + 1):
            nc.scalar.activation(
                out=X0[U:C0, q * 512 : (q + 1) * 512],
                in_=pps[q][0:1, :],
                func=AF.Identity,
                bias=pb_sb[:, 0:1],
                scale=1.0,
            )
        for q in (2 * p, 2 * p + 1):
            nc.scalar.activation(
                out=X0c[U:C0, q * 512 : (q + 1) * 512],
                in_=pps[q][0:1, :],
                func=AF.Identity,
                bias=pb_sb[:, 0:1],
                scale=1.0,
            )

    def run_step(pfx, dec_t=None):
        gs1_0, gs2_0, cs1_0, cs2_0 = cell_phases(
            X0, X0c, C0, 0, X0[0:U, :], X1c[U:C1, :],
            w_sb[f"{pfx}wg0"], w_sb[f"{pfx}bgru0"],
            w_sb[f"{pfx}wc0"], w_sb[f"{pfx}bc0"],
            l0_writer, l0_post,
        )
        gs1_1, gs2_1, cs1_1, cs2_1 = cell_phases(
            X1, X1c, C1, 0, h1t[:, :], X1[U:C1, :],
            w_sb[f"{pfx}wg1"], w_sb[f"{pfx}bgru1"],
            w_sb[f"{pfx}wc1"], w_sb[f"{pfx}bc1"],
            l1_writer, l1_post,
        )
        gs1_0(0); gs1_0(1); gs2_0(0); gs2_0(1)
        cs1_0(0); cs1_0(1); cs2_0(0); cs2_0(1)
        gs1_1(0); gs1_1(1); gs2_1(0); gs2_1(1)
        cs1_1(0); cs1_1(1); cs2_1(0); cs2_1(1)
        if dec_t is not None:
            proj_phase(0)
            proj_phase(1)
            nc.sync.dma_start(out=d_out[dec_t : dec_t + 1, :], in_=X0[U:C0, :])

    # ================= encoder =================
    for t in range(n_enc):
        nc.sync.dma_start(out=X0[U:C0, :], in_=d_xenc[t : t + 1, :])
        nc.sync.dma_start(out=X0c[U:C0, :], in_=d_xenc[t : t + 1, :])
        run_step("e")
        if t == 0:
            # stream decoder weights in during encoder compute; the sync
            # queue is idle from here until the next step's input DMA
            load_weights("d")
            nc.sync.dma_start(out=pw_sb[:, :], in_=d_pw[:, :])
            nc.sync.dma_start(out=pb_sb, in_=d_pb[:, :])

    # ================= decoder =================
    nc.gpsimd.memset(X0[U:C0, :], 0.0)
    nc.gpsimd.memset(X0c[U:C0, :], 0.0)
    for t in range(n_dec):
        run_step("d", dec_t=t)

    for pool in (ps2, ps1, ac_pool, ag_pool, gpool, work, consts):
        pool.release()


# --------------------------------------------------------------------------
# host-side packing
# --------------------------------------------------------------------------
def _prep_shared(inputs):
    import ml_dtypes

    bf16 = ml_dtypes.bfloat16
    sup = np.asarray(inputs["supports"], np.float64)
    eye = np.eye(N, dtype=np.float64)
    tms = [
        sup[0],
        2.0 * (sup[0] @ sup[0]) - eye,
        sup[1],
        2.0 * (sup[1] @ sup[1]) - eye,
    ]
    tmats = np.stack([t.T for t in tms]).astype(np.float32)  # [m-1, j, i]
    tmats = tmats.reshape((NM - 1) * KCH * 128, 512)

    shared = {"tmats": np.ascontiguousarray(tmats.astype(bf16))}
    for pfx, name in (("e", "enc"), ("d", "dec")):
        for lyr, c_in in ((0, C0), (1, C1)):
            wg = np.asarray(inputs[f"{name}{lyr}_Wg"], np.float32)
            wc = np.asarray(inputs[f"{name}{lyr}_Wc"], np.float32)
            wg = wg.reshape(c_in, NM * 2 * U)
            wc = wc.reshape(c_in, NM * U)
            bg = np.asarray(inputs[f"{name}{lyr}_bg"], np.float32)
            bc = np.asarray(inputs[f"{name}{lyr}_bc"], np.float32)
            perm_ur = np.r_[U : 2 * U, 0:U]  # gate out-channels as [u ; r]
            wg_r = wg.reshape(c_in, NM, 2 * U)[:, :, perm_ur].reshape(
                c_in, NM * 2 * U
            )
            wc_r = wc.reshape(c_in, NM, U).reshape(c_in, NM * U)
            if lyr == 0:
                perm = np.r_[1:c_in, 0]  # rows [h..., x]
                wg_r = wg_r[perm]
                wc_r = wc_r[perm]
            else:
                # X1c rows are [r*h1 ; h0]: candidate weight rows follow
                wc_r = wc_r[np.r_[U:c_in, 0:U]]
            shared[f"{pfx}wg{lyr}"] = np.ascontiguousarray(wg_r.astype(bf16))
            shared[f"{pfx}wc{lyr}"] = np.ascontiguousarray(wc_r.astype(bf16))
            shared[f"{pfx}bgru{lyr}"] = np.ascontiguousarray(
                np.concatenate([bg[U:], bg[:U]]).reshape(2 * U, 1)
            )
            shared[f"{pfx}bc{lyr}"] = np.ascontiguousarray(bc.reshape(U, 1))
    pw = np.asarray(inputs["proj_W"], np.float32).reshape(U, 1)
    shared["pw"] = np.ascontiguousarray(
        np.concatenate([pw, np.zeros((U, 1), np.float32)], axis=1).astype(bf16)
    )
    shared["pb"] = np.asarray(inputs["proj_b"], np.float32).reshape(1, 1)
    return shared


def _make_in_maps(inputs, n_enc=T_ENC):
    import ml_dtypes

    shared = _prep_shared(inputs)
    x = np.asarray(inputs["inputs"], np.float32)  # (T, B, N)
    in_maps = []
    for c in range(NCORES):
        m = dict(shared)
        m["xenc"] = np.ascontiguousarray(
            x[:n_enc, c * BL : (c + 1) * BL, :]
            .reshape(n_enc, BI)
            .astype(ml_dtypes.bfloat16)
        )
        in_maps.append(m)
    return in_maps


_PROG_CACHE = {}


def _get_program(n_enc=T_ENC, n_dec=HOR):
    key = (n_enc, n_dec)
    if key not in _PROG_CACHE:
        _PROG_CACHE[key] = _build_program(n_enc, n_dec)
    return _PROG_CACHE[key]


def _run(inputs, n_enc=T_ENC, n_dec=HOR, **kw):
    nc = _get_program(n_enc, n_dec)
    in_maps = _make_in_maps(inputs, n_enc)
    res = bass_utils.run_bass_kernel_spmd(nc, in_maps, core_ids=list(range(NCORES)), **kw)
    out = np.empty((n_dec, B, N), np.float32)
    for c in range(NCORES):
        out[:, c * BL : (c + 1) * BL, :] = (
            res.results[c]["outs"].astype(np.float32).reshape(n_dec, BL, N)
        )
    return out.reshape(n_dec, B, N), res


def kernel(**inputs) -> np.ndarray:
    out, _ = _run(inputs)
    return out.reshape(HOR, B, N)
